# revision 49
# baseline (speedup 1.0000x reference)
"""Causal self-attention (B=2, T=2048, D=1024, H=16) on 8 TRN2 NeuronCores.

Sharding: data-parallel over batch (cores 0-3 -> batch 0, cores 4-7 -> batch 1),
tensor-parallel over heads (4 heads / 256 output dims per core). Each core
computes q/k/v projections for its heads, causal flash-style attention, and a
partial output projection (contraction over its 256 dims of Wo). The host sums
the 4 partials per batch and adds bo.

All matmuls run in bf16 (fp32 PSUM accumulation). Projection / output-
projection matmuls are woven one-at-a-time into the attention tile loop so the
PE never idles waiting for the scalar engine's exp. Inputs are pre-permuted on
the host to [128, chunk, ...] so each tensor loads with a single dma_start
(SP dispatch is ~600ns per DMA and was the old head bottleneck).
"""
import sys

sys.path.insert(0, '/opt/trn_rl_repo')

import numpy as np

import concourse.bass as bass  # noqa: F401  (import keeps bass registered)
import concourse.mybir as mybir
import concourse.tile as tile
from concourse import bacc
from concourse.bass_utils import run_bass_kernel_spmd

F32 = mybir.dt.float32
BF16 = mybir.dt.bfloat16
AF = mybir.ActivationFunctionType

B, T, D, H, HD = 2, 2048, 1024, 16, 64
NCORES = 8
E = 256          # output dims per core (4 heads x 64)
DM = 8           # d_model chunks of 128
TQ = 512
NTQ = T // TQ    # 4
TKT = 128
NTKT = T // TKT  # 16

_CACHE = {}


def _build():
    nc = bacc.Bacc("TRN2", target_bir_lowering=False, debug=False)

    # x is window-major on the host so each query-window DMA reads 8KB
    # contiguous runs per partition (small runs halve effective DMA bandwidth)
    xTd = nc.dram_tensor("xT", [128, NTQ, DM, TQ], BF16, kind="ExternalInput")
    wqd = nc.dram_tensor("wq", [128, DM, E], BF16, kind="ExternalInput")
    wkd = nc.dram_tensor("wk", [128, DM, E], BF16, kind="ExternalInput")
    wvd = nc.dram_tensor("wv", [128, DM, E], BF16, kind="ExternalInput")
    wod = nc.dram_tensor("wo", [128, 2, D], BF16, kind="ExternalInput")
    bq_d = nc.dram_tensor("bq", [E, 1], F32, kind="ExternalInput")
    bk_d = nc.dram_tensor("bk", [E, 1], F32, kind="ExternalInput")
    bvb_d = nc.dram_tensor("bvb", [128, E], F32, kind="ExternalInput")
    onesr_d = nc.dram_tensor("onesr", [33, HD], BF16, kind="ExternalInput")
    outT = nc.dram_tensor("outT", [8, NTQ, 128, TQ], BF16, kind="ExternalOutput")

    with tile.TileContext(nc) as tc, nc.allow_low_precision(reason="bf16 attn"):
        with (
            tc.tile_pool(name="persist", bufs=1) as pp,
            tc.tile_pool(name="xw", bufs=1) as xw,
            tc.tile_pool(name="work", bufs=12) as wk_pool,
            tc.tile_pool(name="ostage", bufs=8) as op_pool,
            tc.tile_pool(name="small", bufs=2) as sm,
            tc.tile_pool(name="psS", bufs=2, space="PSUM") as ps_s,
            tc.tile_pool(name="psY", bufs=1, space="PSUM") as ps_y,
            tc.tile_pool(name="psP", bufs=2, space="PSUM") as ps_p,
        ):
            xb = xw.tile([128, NTQ, DM, TQ], BF16, tag="x", name="x")
            wqb = xw.tile([128, DM, E], BF16, tag="wq", name="wq")
            wkb = xw.tile([128, DM, E], BF16, tag="wk", name="wk")
            wvb = xw.tile([128, DM, E], BF16, tag="wv", name="wv")
            wob = pp.tile([128, 2, D], BF16, tag="wo", name="wo")

            # few big dma_starts: SP dispatch is ~600ns each and packets of one
            # DMA spread across all 16 DMA engines. wq/x-w0 split in halves so
            # the first projection chain starts as soon as its chunks land.
            # split input DMAs across BOTH HW-DGE queues (sync + scalar): two
            # descriptor generators in parallel roughly double input
            # bandwidth. Big transfers first on each queue: dispatch order is
            # completion order, and the tiny bias loads aren't needed until
            # the first bias-add at ~13us.
            # tiny bias loads go via the Pool engine's software DGE: they
            # dispatch at t~0 without occupying either HW-DGE queue
            bq_sb, bk_sb = [], []
            for e2 in range(2):
                t_ = pp.tile([128, 1], F32, tag=f"bq{e2}")
                nc.gpsimd.dma_start(out=t_[:], in_=bq_d[e2 * 128:(e2 + 1) * 128, :])
                bq_sb.append(t_)
                t_ = pp.tile([128, 1], F32, tag=f"bk{e2}")
                nc.gpsimd.dma_start(out=t_[:], in_=bk_d[e2 * 128:(e2 + 1) * 128, :])
                bk_sb.append(t_)
            bvb = pp.tile([128, E], F32, tag="bvb")
            nc.gpsimd.dma_start(out=bvb[:], in_=bvb_d[:, :])
            onesr = pp.tile([33, HD], BF16, tag="onesr")
            nc.gpsimd.dma_start(out=onesr[:], in_=onesr_d[:, :])
            # the scalar (ACT) HW-DGE queue carries only the first-needed
            # halves: every dispatch there delays the first exp by ~667ns
            nc.sync.dma_start(out=wqb[:, 0:4, :], in_=wqd[:, 0:4, :])
            nc.scalar.dma_start(out=wqb[:, 4:8, :], in_=wqd[:, 4:8, :])
            nc.sync.dma_start(out=xb[:, 0, 0:4, :], in_=xTd[:, 0, 0:4, :])
            nc.scalar.dma_start(out=xb[:, 0, 4:8, :], in_=xTd[:, 0, 4:8, :])
            nc.sync.dma_start(out=wkb[:], in_=wkd[:])
            nc.scalar.dma_start(out=wvb[:], in_=wvd[:])
            nc.sync.dma_start(out=xb[:, 1, 0:4, :], in_=xTd[:, 1, 0:4, :])
            nc.scalar.dma_start(out=xb[:, 1, 4:8, :], in_=xTd[:, 1, 4:8, :])
            nc.sync.dma_start(out=xb[:, 2, 0:4, :], in_=xTd[:, 2, 0:4, :])
            nc.sync.dma_start(out=xb[:, 2, 4:8, :], in_=xTd[:, 2, 4:8, :])
            nc.sync.dma_start(out=wob[:], in_=wod[:])
            nc.sync.dma_start(out=xb[:, 3, 0:4, :], in_=xTd[:, 3, 0:4, :])
            nc.sync.dma_start(out=xb[:, 3, 4:8, :], in_=xTd[:, 3, 4:8, :])

            qT_sb = [pp.tile([128, T], BF16, tag=f"qT{i}", name=f"qT{i}") for i in range(2)]
            kT_sb = [pp.tile([128, T], BF16, tag=f"kT{i}", name=f"kT{i}") for i in range(2)]
            v_sb = [pp.tile([128, 4, HD + 1], BF16, tag=f"v{t}", name=f"v{t}")
                    for t in range(NTKT)]
            yT_sb = [pp.tile([128, T], BF16, tag=f"yT{i}", name=f"yT{i}") for i in range(2)]

            # constant ones column of v (denominator trick), set once
            for t in range(NTKT):
                nc.gpsimd.memset(v_sb[t][:, :, HD:HD + 1], 1.0)

            def gen_qk(tq, e2s=(0, 1)):
                """Yields once per matmul; q/k projection for query window tq."""
                for wi, (w_sb, b_sb, dst) in enumerate(
                        ((wqb, bq_sb, qT_sb), (wkb, bk_sb, kT_sb))):
                    for e2 in e2s:
                        pt = ps_p.tile([128, TQ], F32, tag="p",
                                       name=f"ppqk_{tq}_{wi}_{e2}")
                        for c in range(DM):
                            nc.tensor.matmul(
                                pt[:],
                                w_sb[:, c, e2 * 128:(e2 + 1) * 128],
                                xb[:, tq, c, :],
                                start=(c == 0), stop=(c == DM - 1))
                            if c < DM - 1:
                                yield None
                        nc.vector.tensor_scalar_add(
                            out=dst[e2][:, tq * TQ:(tq + 1) * TQ],
                            in0=pt[:], scalar1=b_sb[e2][:])
                        yield True

            def gen_v(trange):
                """Yields once per matmul; v projection for 128-token tiles."""
                for t in trange:
                    pt = ps_p.tile([128, E], F32, tag="p", name=f"ppv_{t}")
                    o = (t % 4) * 128
                    for c in range(DM):
                        nc.tensor.matmul(
                            pt[:],
                            xb[:, t // 4, c, o:o + 128],
                            wvb[:, c, :],
                            start=(c == 0), stop=(c == DM - 1))
                        if c < DM - 1:
                            yield None
                    nc.vector.tensor_add(
                        out=v_sb[t][:, :, 0:HD],
                        in0=pt[:].rearrange("p (h d) -> p h d", h=4),
                        in1=bvb[:].rearrange("p (h d) -> p h d", h=4))
                    yield True

            def gen_oproj(tq_o, nw=1):
                """Yields once per matmul; partial out-proj for window tq_o.

                nw: sub-windows per TQ window (use 2 on the last window so the
                trailing DMA is half as long)."""
                w = TQ // nw
                final = tq_o == NTQ - 1
                for e8 in range(8):
                    for sw in range(nw):
                        q0 = tq_o * TQ + sw * w
                        pt = ps_p.tile([128, w], F32, tag="p",
                                       name=f"poc_{tq_o}_{e8}_{sw}")
                        for d2 in range(2):
                            nc.tensor.matmul(
                                pt[:],
                                wob[:, d2, e8 * 128:(e8 + 1) * 128],
                                yT_sb[d2][:, q0:q0 + w],
                                start=(d2 == 0), stop=(d2 == 1))
                            if d2 == 0:
                                yield None
                        ot = op_pool.tile([128, w], BF16, tag="ostage",
                                          name=f"oto_{tq_o}_{e8}_{sw}")
                        # in the final drain the ACT engine is idle: alternate
                        # the PSUM->SBUF copies and DMA dispatch across
                        # DVE+sync / ACT+ACT so the tail is not DVE-paced
                        if final and (e8 * nw + sw) % 2 == 1:
                            nc.scalar.activation(ot[:], pt[:], AF.Copy)
                            nc.scalar.dma_start(
                                out=outT[e8, tq_o, :, sw * w:(sw + 1) * w],
                                in_=ot[:])
                        else:
                            nc.vector.tensor_copy(out=ot[:], in_=pt[:])
                            nc.sync.dma_start(
                                out=outT[e8, tq_o, :, sw * w:(sw + 1) * w],
                                in_=ot[:])
                        yield True

            def s_stage(tq, pr, tk):
                """S matmuls + exp (+ causal mask for diagonal tiles)."""
                kt = kT_sb[pr]
                qt = qT_sb[pr]
                # diag tiles only need columns >= 128*o (o = tk - 4*tq)
                o = tk - 4 * tq
                c0 = 128 * o if o > 0 else 0
                n = TQ - c0
                ps_t = ps_s.tile([128, 1024], F32, tag="S",
                                 name=f"ps_s_{tq}_{pr}_{tk}")
                q0 = tq * TQ + c0
                nc.tensor.matmul(
                    ps_t[:, c0:TQ],
                    kt[0:64, tk * 128:(tk + 1) * 128],
                    qt[0:64, q0:(tq + 1) * TQ],
                    start=True, stop=True)
                nc.tensor.matmul(
                    ps_t[:, TQ + c0:2 * TQ],
                    kt[64:128, tk * 128:(tk + 1) * 128],
                    qt[64:128, q0:(tq + 1) * TQ],
                    start=True, stop=True)
                es = wk_pool.tile([128, 1024], BF16, tag="expS",
                                  name=f"es_{tq}_{pr}_{tk}")
                if c0 == 0:
                    nc.scalar.activation(es[:], ps_t[:], AF.Exp, scale=0.125)
                elif c0 <= 256:
                    # one contiguous op; the [TQ, TQ+c0) junk span is never read
                    nc.scalar.activation(
                        es[:, c0:2 * TQ], ps_t[:, c0:2 * TQ],
                        AF.Exp, scale=0.125)
                else:
                    for j in range(2):
                        nc.scalar.activation(
                            es[:, j * TQ + c0:(j + 1) * TQ],
                            ps_t[:, j * TQ + c0:(j + 1) * TQ],
                            AF.Exp, scale=0.125)
                if o >= 0:
                    em = wk_pool.tile([128, 1024], BF16, tag="expS",
                                      name=f"em_{tq}_{pr}_{tk}")
                    for j in range(2):
                        nc.gpsimd.affine_select(
                            out=em[:, j * TQ + c0:(j + 1) * TQ],
                            in_=es[:, j * TQ + c0:(j + 1) * TQ],
                            compare_op=mybir.AluOpType.is_ge,
                            fill=0.0,
                            base=0,
                            pattern=[[1, n]],
                            channel_multiplier=-1)
                    es = em
                return es, c0

            def y_stage(tq, pr, tk, py, es, c0, ntk):
                nc.tensor.matmul(
                    py[:, c0:TQ], v_sb[tk][:, 2 * pr, :],
                    es[:, c0:TQ],
                    start=(tk == 0), stop=(tk == ntk - 1))
                nc.tensor.matmul(
                    py[:, TQ + c0:2 * TQ], v_sb[tk][:, 2 * pr + 1, :],
                    es[:, TQ + c0:2 * TQ],
                    start=(tk == 0), stop=(tk == ntk - 1))

            def normalize_a(tq, pr, py):
                """DVE-only reciprocal chain; returns rc for normalize_b.

                Split from normalize_b so filler matmuls can be emitted in
                between: the broadcast matmul waits on rc (DVE), and the PE is
                in-order, so emitting it immediately would head-of-line block
                the PE queue for ~2us."""
                dn = sm.tile([33, TQ], F32, tag="dn", name=f"dn_{tq}_{pr}")
                nc.vector.tensor_copy(out=dn[0:1, :], in_=py[HD:HD + 1, 0:TQ])
                nc.vector.tensor_copy(out=dn[32:33, :], in_=py[HD:HD + 1, TQ:2 * TQ])
                rc32 = sm.tile([33, TQ], F32, tag="rc32", name=f"rc32_{tq}_{pr}")
                nc.vector.reciprocal_approx_fast(out=rc32[:, :], in_=dn[:, :])
                rc = sm.tile([33, TQ], BF16, tag="rc", name=f"rc_{tq}_{pr}")
                nc.vector.tensor_copy(out=rc[:, :], in_=rc32[:, :])
                return rc

            def normalize_b(tq, pr, py, rc):
                # caller must flush any open filler chain first: pb shares
                # the 2-buffer "p" pool, and allocating it while a chain is
                # mid-flight can deadlock the in-order PE queue
                pb = ps_p.tile([128, TQ], F32, tag="p", name=f"pb_{tq}_{pr}")
                for i in range(2):
                    nc.tensor.matmul(pb[i * 64:(i + 1) * 64, :],
                                     onesr[32 * i:32 * i + 1, :],
                                     rc[32 * i:32 * i + 1, :],
                                     start=True, stop=True)
                bc = sm.tile([128, TQ], BF16, tag="bc", name=f"bc_{tq}_{pr}")
                nc.vector.tensor_copy(out=bc[:], in_=pb[:])
                for i in range(2):
                    row0 = i * 64
                    nc.vector.tensor_mul(
                        out=yT_sb[pr][row0:row0 + 64, tq * TQ:(tq + 1) * TQ],
                        in0=py[0:HD, i * TQ:(i + 1) * TQ],
                        in1=bc[row0:row0 + 64, :])

            def drain(g, k=None):
                if g is None:
                    return
                if k is None:
                    for _ in g:
                        pass
                else:
                    for _ in range(k):
                        if next(g, StopIteration) is StopIteration:
                            break

            # ---- schedule ----
            # head: only the pr=0 half of q/k (e2=0) plus v(0..3) gates the
            # first attention group; the e2=1 half rides along as filler
            drain(gen_qk(0, (0,)))
            drain(gen_v(range(0, 4)))
            import itertools

            LEAD = 5  # S tiles emitted ahead of their av consumption
            soft_box = [None, False]  # generator, mid-chain flag

            for tq in range(NTQ):
                ntk = 4 * (tq + 1)
                # crit fillers MUST fully drain within this tq (attention of
                # tq+1 reads their outputs); soft fillers may carry over
                crit_box = [None, False]
                if tq < NTQ - 1:
                    gens = []
                    if tq == 0:
                        gens.append(gen_qk(0, (1,)))  # pr=1 half of window 0
                    gens.append(gen_qk(tq + 1))
                    gens.append(gen_v(range(4 * (tq + 1), 4 * (tq + 1) + 4)))
                    crit_box[0] = itertools.chain(*gens)
                if tq > 0:
                    prev = soft_box[0]
                    nxt = gen_oproj(tq - 1)
                    if tq == NTQ - 1:
                        # oproj(2)'s DVE copies must land before the final
                        # normalizes: drain it ahead of older leftovers
                        soft_box[0] = itertools.chain(nxt, prev or iter(()))
                    else:
                        soft_box[0] = itertools.chain(prev or iter(()), nxt)

                def adv(box):
                    v = next(box[0], StopIteration)
                    if v is StopIteration:
                        box[0] = None
                        box[1] = False
                        return False
                    box[1] = v is not True
                    return True

                def pull(n):
                    for _ in range(n):
                        if crit_box[0] is not None and adv(crit_box):
                            continue
                        if soft_box[0] is not None and adv(soft_box):
                            continue
                        break

                def flush_chains():
                    # close any mid-flight chain so normalize_b's pb can
                    # allocate from the "p" pool without a circular wait
                    for box in (crit_box, soft_box):
                        while box[0] is not None and box[1]:
                            adv(box)

                for pr in range(2):
                    if tq == 0 and pr == 1:
                        # guarantee the e2=1 half of qk(0) (first 16 crit
                        # units) is emitted before this group's S reads it
                        pull(16)
                    py = ps_y.tile([65, 1024], F32, tag="y", name=f"py_{tq}_{pr}")
                    pend = []
                    for tk in range(ntk):
                        pend.append(s_stage(tq, pr, tk))
                        if tk >= LEAD:
                            i = tk - LEAD
                            y_stage(tq, pr, i, py, *pend[i], ntk)
                        if tq == NTQ - 1:
                            pull(1)
                        elif tk >= 4 * tq:
                            pull(2)  # diag tiles add Pool mask latency
                        else:
                            pull(1)
                    for i in range(max(0, ntk - LEAD), ntk):
                        y_stage(tq, pr, i, py, *pend[i], ntk)
                    rc = normalize_a(tq, pr, py)
                    # cover the DVE reciprocal latency with filler before the
                    # PE-side broadcast matmul
                    if tq == 0:
                        pull(4)
                    elif tq < NTQ - 1:
                        pull(8)
                    else:
                        pull(2 if pr == 0 else 6)
                    flush_chains()
                    normalize_b(tq, pr, py, rc)
                    if tq == 0:
                        pull(4)
                    elif tq < NTQ - 1:
                        pull(8)
                    else:
                        pull(4)
                # attention(tq+1) consumes crit outputs: finish them now
                drain(crit_box[0])
                crit_box[0] = None
            drain(soft_box[0])
            drain(gen_oproj(NTQ - 1, nw=2))

    nc.compile()
    return nc


def _get_nc():
    if 'nc' not in _CACHE:
        _CACHE['nc'] = _build()
    return _CACHE['nc']


def _make_in_maps(x, Wq, bq, Wk, bk, Wv, bv, Wo, bo):
    import ml_dtypes
    BF = ml_dtypes.bfloat16
    x = np.asarray(x, dtype=np.float32)
    Wq = np.asarray(Wq, dtype=np.float32)
    Wk = np.asarray(Wk, dtype=np.float32)
    Wv = np.asarray(Wv, dtype=np.float32)
    Wo = np.asarray(Wo, dtype=np.float32)
    bq = np.asarray(bq, dtype=np.float32)
    bk = np.asarray(bk, dtype=np.float32)
    bv = np.asarray(bv, dtype=np.float32)

    onesr = np.ones((33, HD), dtype=BF)

    def chunked(a, nch):
        # [nch*128, m] -> [128, nch, m] so one DMA loads the whole tensor
        m = a.shape[1]
        return np.ascontiguousarray(
            a.reshape(nch, 128, m).transpose(1, 0, 2).astype(BF))

    def xchunked(a):
        # [1024, 2048] -> [128, 4, 8, 512] (window-major: 8KB runs per DMA)
        return np.ascontiguousarray(
            a.reshape(DM, 128, NTQ, TQ).transpose(1, 2, 0, 3).astype(BF))

    in_maps = []
    for c in range(NCORES):
        b, g = divmod(c, 4)
        hs = slice(g * E, (g + 1) * E)
        in_maps.append({
            "xT": xchunked(x[b].T),
            "wq": chunked(Wq[hs].T, DM),
            "wk": chunked(Wk[hs].T, DM),
            "wv": chunked(Wv[hs].T, DM),
            "wo": chunked(Wo[:, hs].T, 2),
            "bq": np.ascontiguousarray(bq[hs].reshape(E, 1)),
            "bk": np.ascontiguousarray(bk[hs].reshape(E, 1)),
            "bvb": np.broadcast_to(bv[hs], (128, E)).copy(),
            "onesr": onesr,
        })
    return in_maps


def kernel(x, Wq, bq, Wk, bk, Wv, bv, Wo, bo, _run_kwargs=None):
    nc = _get_nc()
    in_maps = _make_in_maps(x, Wq, bq, Wk, bk, Wv, bv, Wo, bo)
    last_err = None
    for _attempt in range(3):
        try:
            res = run_bass_kernel_spmd(nc, in_maps, core_ids=list(range(NCORES)),
                                       **(_run_kwargs or {}))
            break
        except Exception as e:  # transient NRT/device hiccups: retry
            last_err = e
            import time as _time
            _time.sleep(2.0)
    else:
        raise last_err
    bo = np.asarray(bo, dtype=np.float32)
    out = np.empty((B, T, D), dtype=np.float32)
    for b in range(B):
        acc = res.results[4 * b]["outT"].astype(np.float32)
        for g in range(1, 4):
            acc += res.results[4 * b + g]["outT"].astype(np.float32)
        # [8, NTQ, 128, TQ] -> [D, T]
        acc = acc.transpose(0, 2, 1, 3).reshape(D, T)
        out[b] = acc.T + bo
    if _run_kwargs:
        _CACHE['last_results'] = res
    return out
